# revision 9
# baseline (speedup 1.0000x reference)
"""Trainium2 Bass kernel for spatial-reduction attention (nn_Attention_11269994184820).

Strategy: head-parallel over 8 cores (8 heads). Each core computes one head's
attention for all 4 batches in a transposed layout (dims on partitions), then an
AllToAll redistributes head-outputs to token-slices, and each core applies the
output projection for its 2048 token rows.

Layouts (per core = head h):
  - xT[b]      [256, 4096]   x transposed (host prep), 2 c-chunks of 128
  - relT       [1024, 4096]  relative_pos[h].T (host prep)
  - qrep       [128, 4096]   qT replicated 4x along partitions (for row-packed QK)
  - kstrip     [128, 128]x2  kT chunks at partition strips (tile_position packing)
  - scores.T   PSUM [128 k, 512 q] = identity-matmul(relT tile) + QK matmul (accumulate)
  - exp        ACT Exp PSUM->SBUF
  - AV         accumulate [v|1] over 8 k-chunks -> [33, 512]: rows 0-31 out.T, row 32 denom
  - AllToAll   [8, 33, 2048] blocks; normalization (x 1/denom) happens after, at proj
"""

import sys

if "/opt/trn_rl_repo" not in sys.path:
    sys.path.insert(0, "/opt/trn_rl_repo")

from contextlib import ExitStack

import numpy as np

import concourse.bacc as bacc
import concourse.bass as bass
import concourse.mybir as mybir
import concourse.tile as tile
from concourse.bass_utils import run_bass_kernel_spmd

F32 = mybir.dt.float32
N_CORES = 8
B, N, C = 4, 4096, 256
HEADS, DH, SR, NK = 8, 32, 2, 1024
BN_EPS = 1e-5

_CACHE = {}


def _build_nc():
    nc = bacc.Bacc("TRN2", target_bir_lowering=False, debug=False, num_devices=N_CORES)

    def din(name, shape):
        return nc.dram_tensor(name, list(shape), F32, kind="ExternalInput").ap()

    xt_d = din("xt", [B, 2, 128, N])
    relt_d = din("relt", [NK, N])
    qw_d = din("qw", [2, 128, 128])
    kw_d = din("kw", [2, 128, 32])
    vw_d = din("vw", [2, 128, 32])
    cw_d = din("cw", [4, 2, 128, 128])
    bna_d = din("bna", [2, 128, 1])
    bnb_d = din("bnb", [2, 128, 1])
    ident_d = din("ident", [128, 128])
    pwt_d = din("pwt", [2, 128, 256])
    pb_d = din("pbrep", [128, 256])
    out_d = nc.dram_tensor("out", [2048, 256], F32, kind="ExternalOutput").ap()

    with tile.TileContext(nc) as tc, ExitStack() as ctx:
        pool = ctx.enter_context(tc.tile_pool(name="main", bufs=1))
        p_dram = ctx.enter_context(tc.tile_pool(name="dram", bufs=1, space="DRAM"))
        ps_sc = ctx.enter_context(tc.tile_pool(name="ps_sc", bufs=2, space="PSUM"))
        ps_av = ctx.enter_context(tc.tile_pool(name="ps_av", bufs=2, space="PSUM"))
        ps_mi = ctx.enter_context(tc.tile_pool(name="ps_mi", bufs=2, space="PSUM"))

        # ---- constants into SBUF ----
        def const_tile(src, shape, tag):
            t = pool.tile(shape, F32, tag=tag)
            nc.sync.dma_start(t[:], src)
            return t

        qw_sb = [const_tile(qw_d[cc], [128, 128], f"qw{cc}") for cc in range(2)]
        kw_sb = [const_tile(kw_d[cc], [128, 32], f"kw{cc}") for cc in range(2)]
        vw_sb = [const_tile(vw_d[cc], [128, 32], f"vw{cc}") for cc in range(2)]
        cw_sb = [[const_tile(cw_d[t, cc], [128, 128], f"cw{t}{cc}") for cc in range(2)]
                 for t in range(4)]
        bna_sb = [const_tile(bna_d[cc], [128, 1], f"bna{cc}") for cc in range(2)]
        bnb_sb = [const_tile(bnb_d[cc], [128, 1], f"bnb{cc}") for cc in range(2)]
        ident_sb = const_tile(ident_d[:], [128, 128], "ident")
        pwt_sb = [const_tile(pwt_d[cc], [128, 256], f"pwt{cc}") for cc in range(2)]
        pb_sb = const_tile(pb_d[:], [128, 256], "pbrep")

        outu_d = p_dram.tile([8, 33, 2048], F32, tag="outu")
        recv_d = p_dram.tile([8, 33, 2048], F32, tag="recv")

        for b in range(B):
            # ---------------- prep: load xT, conv+BN, q/k/v projections ----------
            xt_sb = []
            for cc in range(2):
                t = pool.tile([128, N], F32, tag=f"xt{cc}", bufs=2)
                nc.sync.dma_start(t[:], xt_d[b, cc])
                xt_sb.append(t)

            # depthwise 2x2/2 conv as 4 diag matmuls + BN fold on evacuation
            xkbn = []
            for cc in range(2):
                xk = pool.tile([128, NK], F32, tag=f"xkbn{cc}", bufs=2)
                view = xt_sb[cc][:].rearrange(
                    "p (i a j b) -> p i a j b", i=32, a=2, j=32, b=2
                )
                for half in range(2):
                    psc = ps_mi.tile([128, 512], F32, tag="mi")
                    for t in range(4):
                        di, dj = t // 2, t % 2
                        rhs = view[:, half * 16:(half + 1) * 16, di, :, dj]
                        nc.tensor.matmul(psc[:], cw_sb[t][cc][:], rhs,
                                         start=(t == 0), stop=(t == 3))
                    nc.vector.tensor_scalar(
                        xk[:, half * 512:(half + 1) * 512], psc[:],
                        bna_sb[cc][:], bnb_sb[cc][:],
                        op0=mybir.AluOpType.mult, op1=mybir.AluOpType.add)
                xkbn.append(xk)

            # q projection, output replicated 4x along partitions (qw is pre-tiled)
            qrep = pool.tile([128, N], F32, tag="qrep", bufs=2)
            for ncc in range(8):
                psq = ps_mi.tile([128, 512], F32, tag="mi")
                for cc in range(2):
                    nc.tensor.matmul(psq[:], qw_sb[cc][:],
                                     xt_sb[cc][:, ncc * 512:(ncc + 1) * 512],
                                     start=(cc == 0), stop=(cc == 1))
                nc.vector.tensor_copy(qrep[:, ncc * 512:(ncc + 1) * 512], psq[:])

            # k projection into partition strips: strip s of group g = chunk 4g+s
            kstrip = []
            for grp in range(2):
                psk = ps_mi.tile([128, 128], F32, tag="mi")
                for s in range(4):
                    kc = grp * 4 + s
                    for cc in range(2):
                        nc.tensor.matmul(
                            psk[32 * s:32 * (s + 1), :], kw_sb[cc][:],
                            xkbn[cc][:, kc * 128:(kc + 1) * 128],
                            start=(cc == 0), stop=(cc == 1),
                            tile_position=(0, 32 * s))
                kt = pool.tile([128, 128], F32, tag=f"kstrip{grp}", bufs=2)
                nc.vector.tensor_copy(kt[:], psk[:])
                kstrip.append(kt)

            # v projection in [k-token, d] layout, augmented with ones column
            vsb = []
            for kc in range(8):
                psv = ps_mi.tile([128, 32], F32, tag="mi")
                for cc in range(2):
                    nc.tensor.matmul(psv[:],
                                     xkbn[cc][:, kc * 128:(kc + 1) * 128],
                                     vw_sb[cc][:],
                                     start=(cc == 0), stop=(cc == 1))
                vt = pool.tile([128, 33], F32, tag=f"v{kc}", bufs=2)
                nc.vector.tensor_copy(vt[:, 0:32], psv[:])
                nc.vector.memset(vt[:, 32:33], 1.0)
                vsb.append(vt)

            # ---------------- attention over 8 q-chunks of 512 ----------------
            for qc in range(8):
                psav = ps_av.tile([33, 512], F32, tag="av")
                for g in range(4):  # k-chunk groups of 2
                    pssc = ps_sc.tile([128, 1024], F32, tag="sc")
                    for u in range(2):
                        kc = 2 * g + u
                        rt = pool.tile([128, 512], F32, tag="rel", bufs=6)
                        nc.sync.dma_start(
                            rt[:],
                            relt_d[kc * 128:(kc + 1) * 128,
                                   qc * 512:(qc + 1) * 512])
                        bank = pssc[:, u * 512:(u + 1) * 512]
                        nc.tensor.matmul(bank, ident_sb[:], rt[:],
                                         start=True, stop=False)
                        s = kc % 4
                        nc.tensor.matmul(
                            bank,
                            kstrip[kc // 4][32 * s:32 * (s + 1), :],
                            qrep[32 * s:32 * (s + 1), qc * 512:(qc + 1) * 512],
                            start=False, stop=True, tile_position=(32 * s, 0))
                    et = pool.tile([128, 1024], F32, tag="expt", bufs=3)
                    nc.scalar.activation(et[:], pssc[:],
                                         mybir.ActivationFunctionType.Exp)
                    for u in range(2):
                        kc = 2 * g + u
                        nc.tensor.matmul(psav[:], vsb[kc][:],
                                         et[:, u * 512:(u + 1) * 512],
                                         start=(kc == 0), stop=(kc == 7))
                ou = pool.tile([33, 512], F32, tag="outu", bufs=2)
                nc.vector.tensor_copy(ou[:], psav[:])
                dest = b * 2 + qc // 4
                off = (qc % 4) * 512
                nc.sync.dma_start(outu_d[dest, :, off:off + 512], ou[:])

        # ---------------- exchange head-outputs for token-slices ----------------
        nc.gpsimd.collective_compute(
            "AllToAll", mybir.AluOpType.bypass,
            replica_groups=[list(range(N_CORES))],
            ins=[outu_d.opt()], outs=[recv_d.opt()])

        # ---------------- normalize + output projection for 2048 rows ----------
        den = pool.tile([8, 2048], F32, tag="den")
        for s in range(8):
            nc.sync.dma_start(den[s:s + 1, :], recv_d[s, 32:33, :])
        recip = pool.tile([8, 2048], F32, tag="recip")
        nc.vector.reciprocal(recip[:], den[:])
        recip_d = p_dram.tile([8, 2048], F32, tag="recip_d")
        nc.sync.dma_start(recip_d[:], recip[:])

        lhs = [pool.tile([128, 2048], F32, tag=f"lhs{i}", name=f"lhs{i}")
               for i in range(2)]
        for s in range(8):
            nc.sync.dma_start(lhs[s // 4][(s % 4) * 32:(s % 4 + 1) * 32, :],
                              recv_d[s, 0:32, :])
        bcr = [pool.tile([128, 2048], F32, tag=f"bcr{i}", name=f"bcr{i}")
               for i in range(2)]
        for s in range(8):
            nc.gpsimd.dma_start(
                bcr[s // 4][(s % 4) * 32:(s % 4 + 1) * 32, :],
                recip_d[s, :].partition_broadcast(32))
        for i in range(2):
            nc.vector.tensor_tensor(lhs[i][:], lhs[i][:], bcr[i][:],
                                    op=mybir.AluOpType.mult)

        for r in range(16):
            psp = ps_mi.tile([128, 256], F32, tag="mi")
            for i in range(2):
                nc.tensor.matmul(psp[:], lhs[i][:, r * 128:(r + 1) * 128],
                                 pwt_sb[i][:], start=(i == 0), stop=(i == 1))
            ot = pool.tile([128, 256], F32, tag="ot", bufs=2)
            nc.vector.tensor_add(ot[:], psp[:], pb_sb[:])
            nc.sync.dma_start(out_d[r * 128:(r + 1) * 128, :], ot[:])

    nc.compile()
    return nc


def _host_prep(x, relative_pos, q_w, k_w, v_w, proj_w, proj_b, sr_w, sr_b,
               bn_gamma, bn_beta, bn_mean, bn_var):
    f = np.float32
    x = np.asarray(x, f)
    relative_pos = np.asarray(relative_pos, f)
    scale = np.float32(DH ** -0.5)

    xt = np.ascontiguousarray(x.transpose(0, 2, 1)).reshape(B, 2, 128, N)
    a = (np.asarray(bn_gamma, f) / np.sqrt(np.asarray(bn_var, f) + BN_EPS)).astype(f)
    b_eff = ((np.asarray(sr_b, f) - np.asarray(bn_mean, f)) * a
             + np.asarray(bn_beta, f)).astype(f)
    sr_w = np.asarray(sr_w, f)
    cw = np.zeros((4, 2, 128, 128), f)
    for t in range(4):
        tap = sr_w[:, 0, t // 2, t % 2]
        for cc in range(2):
            cw[t, cc] = np.diag(tap[cc * 128:(cc + 1) * 128])
    bna = a.reshape(2, 128, 1).astype(f)
    bnb = b_eff.reshape(2, 128, 1).astype(f)
    ident = np.eye(128, dtype=f)
    pwt = np.ascontiguousarray(np.asarray(proj_w, f).T).reshape(2, 128, 256)
    pbrep = np.tile(np.asarray(proj_b, f).reshape(1, 256), (128, 1))

    in_maps = []
    for h in range(N_CORES):
        qwT_rep = np.tile(
            np.ascontiguousarray((np.asarray(q_w, f)[h * 32:(h + 1) * 32, :]
                                  * scale).T), (1, 4)).reshape(2, 128, 128)
        kwT = np.ascontiguousarray(
            np.asarray(k_w, f)[h * 32:(h + 1) * 32, :].T).reshape(2, 128, 32)
        vwT = np.ascontiguousarray(
            np.asarray(v_w, f)[h * 32:(h + 1) * 32, :].T).reshape(2, 128, 32)
        relT = np.ascontiguousarray(relative_pos[h].T)
        in_maps.append({
            "xt": xt, "relt": relT, "qw": np.ascontiguousarray(qwT_rep),
            "kw": kwT, "vw": vwT, "cw": cw, "bna": bna, "bnb": bnb,
            "ident": ident, "pwt": np.ascontiguousarray(pwt),
            "pbrep": np.ascontiguousarray(pbrep),
        })
    return in_maps


def run_once(inputs, trace=False, trace_kwargs=None):
    if trace:
        try:
            import antenv.axon_hooks  # noqa: F401
        except ImportError:
            trace = False
    if "nc" not in _CACHE:
        _CACHE["nc"] = _build_nc()
    nc = _CACHE["nc"]
    in_maps = _host_prep(
        inputs["x"], inputs["relative_pos"], inputs["q_w"], inputs["k_w"],
        inputs["v_w"], inputs["proj_w"], inputs["proj_b"], inputs["sr_w"],
        inputs["sr_b"], inputs["bn_gamma"], inputs["bn_beta"],
        inputs["bn_mean"], inputs["bn_var"])
    res = run_bass_kernel_spmd(nc, in_maps, core_ids=list(range(N_CORES)),
                               trace=trace, **(trace_kwargs or {}))
    out = np.zeros((B, N, C), np.float32)
    for i in range(N_CORES):
        bb, nh = i // 2, i % 2
        out[bb, nh * 2048:(nh + 1) * 2048, :] = res.results[i]["out"]
    return out, res


def kernel(**inputs) -> np.ndarray:
    out, _ = run_once(inputs, trace=False)
    return out


# revision 36
# speedup vs baseline: 96.6838x; 96.6838x over previous
"""Trainium2 Bass kernel for spatial-reduction attention (nn_Attention_11269994184820).

Strategy: head-parallel over 8 cores (8 heads). Each core computes one head's
attention for all 4 batches in a transposed layout (dims on partitions), then an
AllToAll redistributes head-outputs to token-slices, and each core applies the
output projection for its 2048 token rows.

Layouts (per core = head h):
  - xT[b]      [256, 4096]   x transposed (host prep), 2 c-chunks of 128
  - relT       [1024, 4096]  relative_pos[h].T (host prep)
  - qrep       [128, 4096]   qT replicated 4x along partitions (for row-packed QK)
  - kstrip     [128, 128]x2  kT chunks at partition strips (tile_position packing)
  - scores.T   PSUM [128 k, 512 q] = identity-matmul(relT tile) + QK matmul (accumulate)
  - exp        ACT Exp PSUM->SBUF
  - AV         accumulate [v|1] over 8 k-chunks -> [33, 512]: rows 0-31 out.T, row 32 denom
  - AllToAll   [8, 33, 2048] blocks; normalization (x 1/denom) happens after, at proj
"""

import sys

if "/opt/trn_rl_repo" not in sys.path:
    sys.path.insert(0, "/opt/trn_rl_repo")

from contextlib import ExitStack

import numpy as np

import concourse.bacc as bacc
import concourse.bass as bass
import concourse.mybir as mybir
import concourse.tile as tile
from concourse.bass_utils import run_bass_kernel_spmd

F32 = mybir.dt.float32
F32R = mybir.dt.float32r
BF16 = mybir.dt.bfloat16
FP16 = mybir.dt.float16
N_CORES = 8
B, N, C = 4, 4096, 256
HEADS, DH, SR, NK = 8, 32, 2, 1024
BN_EPS = 1e-5

# k-chunks whose rel_pos add runs as an fp32 identity-matmul on TensorE;
# the rest are added on VectorE (tensor_tensor from PSUM). Balances PE vs DVE.
PE_ADD_CHUNKS = frozenset({0, 1, 2, 3})

_CACHE = {}


def _r(ap):
    """Reinterpret an fp32 AP as float32r for single-pass (TF32-like) matmul."""
    return ap.bitcast(F32R)


def _build_nc():
    nc = bacc.Bacc("TRN2", target_bir_lowering=False, debug=False, num_devices=N_CORES)

    def din(name, shape, dt=F32):
        return nc.dram_tensor(name, list(shape), dt, kind="ExternalInput").ap()

    xt_d = din("xt", [B, 2, 128, N], F32R)
    relt_d = din("relt", [NK, N])
    qw_d = din("qw", [2, 128, 128], F32R)
    kw_d = din("kw", [2, 128, 32], F32R)
    vw_d = din("vw", [2, 128, 32], F32R)
    cw_d = din("cw", [4, 2, 128, 128], F32R)
    bna_d = din("bna", [2, 128, 1])
    bnb_d = din("bnb", [2, 128, 1])
    ident_d = din("ident", [128, 128])
    pwt_d = din("pwt", [2, 128, 256], F32R)
    pb_d = din("pbrep", [128, 256])
    out_d = nc.dram_tensor("out", [2048, 256], F32, kind="ExternalOutput").ap()

    with tile.TileContext(nc) as tc, ExitStack() as ctx:
        pool = ctx.enter_context(tc.tile_pool(name="main", bufs=1))
        p_dram = ctx.enter_context(tc.tile_pool(name="dram", bufs=1, space="DRAM"))
        ps_sc = ctx.enter_context(tc.tile_pool(name="ps_sc", bufs=2, space="PSUM"))
        ps_av = ctx.enter_context(tc.tile_pool(name="ps_av", bufs=2, space="PSUM"))
        ps_mi = ctx.enter_context(tc.tile_pool(name="ps_mi", bufs=2, space="PSUM"))

        # ---- constants into SBUF ----
        def const_tile(src, shape, tag, dt=F32):
            t = pool.tile(shape, dt, tag=tag)
            nc.sync.dma_start(t[:], src)
            return t

        qw_sb = [const_tile(qw_d[cc], [128, 128], f"qw{cc}", F32R) for cc in range(2)]
        kw_sb = [const_tile(kw_d[cc], [128, 32], f"kw{cc}", F32R) for cc in range(2)]
        vw_sb = [const_tile(vw_d[cc], [128, 32], f"vw{cc}", F32R) for cc in range(2)]
        cw_sb = [[const_tile(cw_d[t, cc], [128, 128], f"cw{t}{cc}", F32R)
                  for cc in range(2)] for t in range(4)]
        bna_sb = [const_tile(bna_d[cc], [128, 1], f"bna{cc}") for cc in range(2)]
        bnb_sb = [const_tile(bnb_d[cc], [128, 1], f"bnb{cc}") for cc in range(2)]
        ident_sb = const_tile(ident_d[:], [128, 128], "ident")
        pwt_sb = [const_tile(pwt_d[cc], [128, 256], f"pwt{cc}", F32R)
                  for cc in range(2)]
        pb_sb = const_tile(pb_d[:], [128, 256], "pbrep")

        outu_d = p_dram.tile([8, 33, 2048], F32, tag="outu")
        recv_d = p_dram.tile([8, 33, 2048], F32, tag="recv")

        def prep_batch(b):
            par = b % 2
            xt_sb = []
            for cc in range(2):
                t = pool.tile([128, N], F32R, tag=f"xt{cc}", bufs=2,
                              name=f"xt{b}{cc}")
                nc.sync.dma_start(t[:], xt_d[b, cc])
                xt_sb.append(t)

            # depthwise 2x2/2 conv as 4 diag matmuls + BN fold on evacuation
            xkbn = []
            for cc in range(2):
                xk = pool.tile([128, NK], F32, tag=f"xkbn{cc}", bufs=2,
                               name=f"xkbn{b}{cc}")
                view = xt_sb[cc][:].rearrange(
                    "p (i a j b) -> p i a j b", i=32, a=2, j=32, b=2
                )
                for half in range(2):
                    psc = ps_mi.tile([128, 512], F32, tag="mi", name=f"cv{b}{cc}{half}")
                    for t in range(4):
                        di, dj = t // 2, t % 2
                        rhs = view[:, half * 16:(half + 1) * 16, di, :, dj]
                        nc.tensor.matmul(psc[:], cw_sb[t][cc][:].bitcast(F32),
                                         rhs.bitcast(F32),
                                         start=(t == 0), stop=(t == 3))
                    nc.vector.tensor_scalar(
                        xk[:, half * 512:(half + 1) * 512], psc[:],
                        bna_sb[cc][:], bnb_sb[cc][:],
                        op0=mybir.AluOpType.mult, op1=mybir.AluOpType.add)
                xkbn.append(xk)

            # q projection (replicated 4x along partitions), bf16 output
            qrep = pool.tile([128, N], FP16, tag=f"qrep{par}", bufs=2,
                             name=f"qrep{b}")
            for ncc in range(8):
                psq = ps_mi.tile([128, 512], F32, tag="mi", name=f"q{b}{ncc}")
                for cc in range(2):
                    nc.tensor.matmul(psq[:], qw_sb[cc][:],
                                     xt_sb[cc][:, ncc * 512:(ncc + 1) * 512],
                                     start=(cc == 0), stop=(cc == 1))
                nc.vector.tensor_copy(qrep[:, ncc * 512:(ncc + 1) * 512], psq[:])

            # k projection into partition strips (bf16)
            kstrip = []
            for grp in range(2):
                psk = ps_mi.tile([128, 128], F32, tag="mi", name=f"k{b}{grp}")
                for s in range(4):
                    kc = grp * 4 + s
                    for cc in range(2):
                        nc.tensor.matmul(
                            psk[32 * s:32 * (s + 1), :],
                            kw_sb[cc][:].bitcast(F32),
                            xkbn[cc][:, kc * 128:(kc + 1) * 128],
                            start=(cc == 0), stop=(cc == 1),
                            tile_position=(0, 32 * s))
                kt = pool.tile([128, 128], FP16, tag=f"ks{par}{grp}", bufs=2,
                               name=f"ks{b}{grp}")
                nc.vector.tensor_copy(kt[:], psk[:])
                kstrip.append(kt)

            # v projection in [k-token, d] layout, + ones column (fp32r memory)
            vsb = []
            for kc in range(8):
                psv = ps_mi.tile([128, 32], F32, tag="mi", name=f"v{b}{kc}")
                for cc in range(2):
                    nc.tensor.matmul(
                        psv[:],
                        xkbn[cc][:, kc * 128:(kc + 1) * 128],
                        vw_sb[cc][:].bitcast(F32),
                        start=(cc == 0), stop=(cc == 1))
                vt = pool.tile([128, 33], FP16, tag=f"v{par}{kc}", bufs=2,
                               name=f"vt{b}{kc}")
                nc.vector.tensor_copy(vt[:, 0:32], psv[:])
                nc.vector.memset(vt[:, 32:33], 1.0)
                vsb.append(vt)
            return qrep, kstrip, vsb

        # batches processed in pairs so each rel_pos tile is loaded once per pair
        for pair in range(2):
            bctx = [prep_batch(2 * pair), prep_batch(2 * pair + 1)]
            for qc in range(8):
                av = [ps_av.tile([33, 512], F32, tag="av", name=f"av{pair}{qc}{i}")
                      for i in range(2)]
                for half in range(2):  # 4 k-chunks per half
                    rts = []
                    for g2 in range(2):
                        g = half * 2 + g2
                        rt = pool.tile([128, 1024], F32, tag="rel", bufs=4,
                                       name=f"rt{pair}{qc}{g}")
                        for u in range(2):
                            kc = 2 * g + u
                            nc.sync.dma_start(
                                rt[:, u * 512:(u + 1) * 512],
                                relt_d[kc * 128:(kc + 1) * 128,
                                       qc * 512:(qc + 1) * 512])
                        rts.append(rt)
                    if half == 0:
                        # rel added on TensorE via fp32 identity-matmul
                        for g2 in range(2):
                            g = half * 2 + g2
                            for ib in range(2):
                                qrep, kstrip, vsb = bctx[ib]
                                pssc = ps_sc.tile([128, 1024], F32, tag="sc",
                                                  name=f"sc{pair}{qc}{g}{ib}")
                                for u in range(2):
                                    kc = 2 * g + u
                                    s = kc % 4
                                    nc.tensor.matmul(
                                        pssc[:, u * 512:(u + 1) * 512],
                                        ident_sb[:],
                                        rts[g2][:, u * 512:(u + 1) * 512],
                                        start=True, stop=False)
                                    nc.tensor.matmul(
                                        pssc[:, u * 512:(u + 1) * 512],
                                        kstrip[kc // 4][32 * s:32 * (s + 1), :],
                                        qrep[32 * s:32 * (s + 1),
                                             qc * 512:(qc + 1) * 512],
                                        start=False, stop=True,
                                        tile_position=(32 * s, 0))
                                et2 = pool.tile([128, 1024], FP16, tag="expt2",
                                                bufs=2,
                                                name=f"et2{pair}{qc}{g}{ib}")
                                nc.scalar.activation(
                                    et2[:], pssc[:],
                                    mybir.ActivationFunctionType.Exp)
                                for u in range(2):
                                    kc = 2 * g + u
                                    nc.tensor.matmul(
                                        av[ib][:], vsb[kc][:],
                                        et2[:, u * 512:(u + 1) * 512],
                                        start=(kc == 0), stop=(kc == 7))
                        continue
                    if half == 0:
                        for g2 in range(2):
                            g = half * 2 + g2
                            for ib in range(2):
                                qrep, kstrip, vsb = bctx[ib]
                                pssc = ps_sc.tile([128, 1024], F32, tag="sc",
                                                  name=f"sc{pair}{qc}{g}{ib}")
                                for u in range(2):
                                    kc = 2 * g + u
                                    s = kc % 4
                                    nc.tensor.matmul(
                                        pssc[:, u * 512:(u + 1) * 512],
                                        ident_sb[:],
                                        rts[g2][:, u * 512:(u + 1) * 512],
                                        start=True, stop=False)
                                    nc.tensor.matmul(
                                        pssc[:, u * 512:(u + 1) * 512],
                                        kstrip[kc // 4][32 * s:32 * (s + 1), :],
                                        qrep[32 * s:32 * (s + 1),
                                             qc * 512:(qc + 1) * 512],
                                        start=False, stop=True,
                                        tile_position=(32 * s, 0))
                                et2 = pool.tile([128, 1024], FP16, tag="expt2",
                                                bufs=2,
                                                name=f"et2{pair}{qc}{g}{ib}")
                                nc.scalar.activation(
                                    et2[:], pssc[:],
                                    mybir.ActivationFunctionType.Exp)
                                for u in range(2):
                                    kc = 2 * g + u
                                    nc.tensor.matmul(
                                        av[ib][:], vsb[kc][:],
                                        et2[:, u * 512:(u + 1) * 512],
                                        start=(kc == 0), stop=(kc == 7))
                        continue
                    ssbs = [pool.tile([128, 2048], F32, tag="ssb", bufs=3,
                                      name=f"ssb{pair}{qc}{half}{ib}")
                            for ib in range(2)]
                    for g2 in range(2):
                        g = half * 2 + g2
                        for ib in range(2):
                            qrep, kstrip, vsb = bctx[ib]
                            pssc = ps_sc.tile([128, 1024], F32, tag="sc",
                                              name=f"sc{pair}{qc}{g}{ib}")
                            for u in range(2):
                                kc = 2 * g + u
                                s = kc % 4
                                nc.tensor.matmul(
                                    pssc[:, u * 512:(u + 1) * 512],
                                    kstrip[kc // 4][32 * s:32 * (s + 1), :],
                                    qrep[32 * s:32 * (s + 1),
                                         qc * 512:(qc + 1) * 512],
                                    start=True, stop=True,
                                    tile_position=(32 * s, 0))
                            nc.vector.tensor_add(
                                ssbs[ib][:, g2 * 1024:(g2 + 1) * 1024],
                                pssc[:], rts[g2][:])
                    for ib in range(2):
                        vsb = bctx[ib][2]
                        et = pool.tile([128, 2048], FP16, tag="expt", bufs=2,
                                       name=f"et{pair}{qc}{half}{ib}")
                        nc.scalar.activation(et[:], ssbs[ib][:],
                                             mybir.ActivationFunctionType.Exp)
                        for j in range(4):
                            kc = half * 4 + j
                            nc.tensor.matmul(av[ib][:], vsb[kc][:],
                                             et[:, j * 512:(j + 1) * 512],
                                             start=(kc == 0), stop=(kc == 7))
                for ib in range(2):
                    b = 2 * pair + ib
                    ou = pool.tile([33, 512], F32, tag="outu", bufs=4,
                                   name=f"ou{pair}{qc}{ib}")
                    nc.scalar.copy(ou[:], av[ib][:])
                    dest = b * 2 + qc // 4
                    off = (qc % 4) * 512
                    nc.sync.dma_start(outu_d[dest, :, off:off + 512], ou[:])

        # ---------------- exchange head-outputs for token-slices ----------------
        nc.gpsimd.collective_compute(
            "AllToAll", mybir.AluOpType.bypass,
            replica_groups=[list(range(N_CORES))],
            ins=[outu_d.opt()], outs=[recv_d.opt()])

        # ---------------- normalize + output projection for 2048 rows ----------
        # denominators packed [128, 128]: head s -> partitions 16s..16s+16
        den = pool.tile([128, 128], F32, tag="den")
        for s in range(8):
            nc.sync.dma_start(den[16 * s:16 * (s + 1), :], recv_d[s, 32:33, :])
        recip = pool.tile([128, 128], F32, tag="recip")
        nc.vector.reciprocal(recip[:], den[:])
        recip_d = p_dram.tile([8, 2048], F32, tag="recip_d")
        for s in range(8):
            nc.sync.dma_start(recip_d[s, :], recip[16 * s:16 * (s + 1), :])

        lhs = [pool.tile([128, 2048], F32, tag=f"qrep{i}", name=f"lhs{i}", bufs=2)
               for i in range(2)]
        for s in range(8):
            nc.sync.dma_start(lhs[s // 4][(s % 4) * 32:(s % 4 + 1) * 32, :],
                              recv_d[s, 0:32, :])
        bcr = [pool.tile([128, 2048], F32, tag=f"xt{i}", name=f"bcr{i}", bufs=2)
               for i in range(2)]
        for s in range(8):
            nc.gpsimd.dma_start(
                bcr[s // 4][(s % 4) * 32:(s % 4 + 1) * 32, :],
                recip_d[s, :].partition_broadcast(32))
        for i in range(2):
            nc.vector.tensor_tensor(lhs[i][:], lhs[i][:], bcr[i][:],
                                    op=mybir.AluOpType.mult)
        for r in range(16):
            psp = ps_mi.tile([128, 256], F32, tag="mi")
            for i in range(2):
                nc.tensor.matmul(psp[:], lhs[i][:, r * 128:(r + 1) * 128],
                                 pwt_sb[i][:].bitcast(F32),
                                 start=(i == 0), stop=(i == 1))
            ot = pool.tile([128, 256], F32, tag="ot", bufs=2)
            nc.vector.tensor_add(ot[:], psp[:], pb_sb[:])
            nc.sync.dma_start(out_d[r * 128:(r + 1) * 128, :], ot[:])

    nc.compile()
    return nc


def _host_prep(x, relative_pos, q_w, k_w, v_w, proj_w, proj_b, sr_w, sr_b,
               bn_gamma, bn_beta, bn_mean, bn_var):
    f = np.float32
    x = np.asarray(x, f)
    relative_pos = np.asarray(relative_pos, f)
    scale = np.float32(DH ** -0.5)

    xt = np.ascontiguousarray(x.transpose(0, 2, 1)).reshape(B, 2, 128, N)
    a = (np.asarray(bn_gamma, f) / np.sqrt(np.asarray(bn_var, f) + BN_EPS)).astype(f)
    b_eff = ((np.asarray(sr_b, f) - np.asarray(bn_mean, f)) * a
             + np.asarray(bn_beta, f)).astype(f)
    sr_w = np.asarray(sr_w, f)
    cw = np.zeros((4, 2, 128, 128), f)
    for t in range(4):
        tap = sr_w[:, 0, t // 2, t % 2]
        for cc in range(2):
            cw[t, cc] = np.diag(tap[cc * 128:(cc + 1) * 128])
    bna = a.reshape(2, 128, 1).astype(f)
    bnb = b_eff.reshape(2, 128, 1).astype(f)
    ident = np.eye(128, dtype=f)
    pwt = np.ascontiguousarray(np.asarray(proj_w, f).T).reshape(2, 128, 256)
    pbrep = np.tile(np.asarray(proj_b, f).reshape(1, 256), (128, 1))

    in_maps = []
    for h in range(N_CORES):
        qwT_rep = np.tile(
            np.ascontiguousarray((np.asarray(q_w, f)[h * 32:(h + 1) * 32, :]
                                  * scale).T), (1, 4)).reshape(2, 128, 128)
        kwT = np.ascontiguousarray(
            np.asarray(k_w, f)[h * 32:(h + 1) * 32, :].T).reshape(2, 128, 32)
        vwT = np.ascontiguousarray(
            np.asarray(v_w, f)[h * 32:(h + 1) * 32, :].T).reshape(2, 128, 32)
        relT = np.ascontiguousarray(relative_pos[h].T)
        in_maps.append({
            "xt": xt, "relt": relT, "qw": np.ascontiguousarray(qwT_rep),
            "kw": kwT, "vw": vwT, "cw": cw, "bna": bna, "bnb": bnb,
            "ident": ident, "pwt": np.ascontiguousarray(pwt),
            "pbrep": np.ascontiguousarray(pbrep),
        })
    return in_maps


def run_once(inputs, trace=False, trace_kwargs=None):
    if trace:
        try:
            import antenv.axon_hooks  # noqa: F401
        except ImportError:
            trace = False
    if "nc" not in _CACHE:
        _CACHE["nc"] = _build_nc()
    nc = _CACHE["nc"]
    in_maps = _host_prep(
        inputs["x"], inputs["relative_pos"], inputs["q_w"], inputs["k_w"],
        inputs["v_w"], inputs["proj_w"], inputs["proj_b"], inputs["sr_w"],
        inputs["sr_b"], inputs["bn_gamma"], inputs["bn_beta"],
        inputs["bn_mean"], inputs["bn_var"])
    res = run_bass_kernel_spmd(nc, in_maps, core_ids=list(range(N_CORES)),
                               trace=trace, **(trace_kwargs or {}))
    out = np.zeros((B, N, C), np.float32)
    for i in range(N_CORES):
        bb, nh = i // 2, i % 2
        out[bb, nh * 2048:(nh + 1) * 2048, :] = res.results[i]["out"]
    return out, res


def kernel(**inputs) -> np.ndarray:
    out, _ = run_once(inputs, trace=False)
    return out
